# revision 23
# baseline (speedup 1.0000x reference)
"""MixedScoreMultiHeadAttention on 8 TRN2 NeuronCores.

Sharding: data-parallel over batch B=8 (one batch element per core, no
collectives).  Per core (R=C=256, E=512, H=8, D=64, HID=128):

  1. QKV projections (bf16 matmuls; all inputs host-packed into the exact
     SBUF tile layouts so every load is one large contiguous DMA).
  2. Per-head dot scores (K=64 matmuls, 2 heads packed via row groups),
     evicted in [128, 512] head-pairs, then DRAM-bounced channel-major
     into S4 [32g+ch, (a, c)] for the score-MLP (K=9).
  3. Wave loop over 32 units of 2048 positions, software-pipelined one
     unit deep (iteration k issues W1+relu(k) then W2+exp(k-1)): W1 = 4
     concurrent row-tiled matmuls, each filling a full 512-f32 PSUM bank
     (concurrent matmuls must never share a bank); relu evict is the
     elementwise bottleneck, split 1 ACT instr [0:896] + 2 DVE instrs to
     amortize ACT's ~250ns access bubble; W2 = 4 concurrent col-tiled
     matmuls, zero-padded to M=32 so each writes its full partition band
     (no uninitialized-PSUM reads); exp is fused into the W2-PSUM evict
     so the logit bounce carries bf16 exp'd weights and no separate exp
     or softmax-numerator pass exists.  w1ps bufs=3 gives each relu >1
     unit before its PSUM slot is reused by a later W1.
  4. Bounce eb -> mb DRAM [r, h, c] (Pool/SWDGE + Sync issues; gather
     side reads 4KB contiguous runs) -> l_sb [r, (h, c)].  Softmax
     denominator via one fused DVE scalar_tensor_tensor per head
     (mask-mul + accumulated row-sum), reciprocal on DVE, per-head
     scaling split ACT/DVE (GPSIMD is ~15x too slow for elementwise and
     has no PSUM port).  PE transposes (batched 4 per PSUM bank) -> AV
     (16 matmuls into one bank, single evict) -> out projection, all
     interleaved into the other row-chunk's wave loop.

The score-MLP weights are algebraically folded on the host:
  hidden = relu(concat_h[dot_h, alpha_h*cost] @ W1)
         = relu(sum_h dot_h * W1[2h,:] + cost * sum_h alpha_h W1[2h+1,:])
so the device sees a 9-channel input (8 raw-dot channels + 1 cost channel)
and an M9 [9, HID] matrix with the 1/sqrt(D) norm folded into the dot rows.
Softmax needs no max-subtraction (logits are provably O(5)); fully-masked
rows fall out via +eps on the denominator with exactly-zero weights.
"""

import os

os.environ.setdefault("MYCRO_LOCAL_CACHE", "1")

import numpy as np
import ml_dtypes

import concourse.bass as bass
import concourse.mybir as mybir
import concourse.tile as tile
from concourse import bacc
from concourse.bass_utils import run_bass_kernel_spmd
from concourse.masks import make_identity

try:  # best-effort NTFF profiling hook (axon image lacks it by default)
    from antenv.axon_hooks import (
        get_axon_ntff_profile_hook,
        set_axon_ntff_profile_hook,
    )

    if get_axon_ntff_profile_hook() is None:
        from trn_agent_boot.trn_boot import _ntff_profile_via_ctypes

        set_axon_ntff_profile_hook(
            _ntff_profile_via_ctypes("/opt/axon/libaxon_pjrt.so")
        )
except Exception:
    pass

BF16 = mybir.dt.bfloat16
F32 = mybir.dt.float32
AF = mybir.ActivationFunctionType
ALU = mybir.AluOpType

B, R, C, E = 8, 256, 256, 512
H, D, HID = 8, 64, 128
NCORES = 8
XA = 896  # relu split: ACT takes wpsA[0:XA]; DVE the rest + all of wpsB

LAST_EXEC_NS = None
_CACHE = {}


def _build():
    nc = bacc.Bacc(
        "TRN2", target_bir_lowering=False, debug=False, enable_asserts=False
    )
    t = {}
    # host-packed inputs: [rows, cols] -> [128, (rows//128, cols)] so each
    # load is a single DMA with 128 fully-contiguous per-partition runs
    t["remb"] = nc.dram_tensor("remb", [128, 4 * R], BF16, kind="ExternalInput")
    t["cemb"] = nc.dram_tensor("cemb", [128, 4 * C], BF16, kind="ExternalInput")
    for w in ("wq", "wk", "wv", "wo"):
        t[w] = nc.dram_tensor(w, [128, 4 * E], BF16, kind="ExternalInput")
    t["m9"] = nc.dram_tensor("m9", [128, HID], BF16, kind="ExternalInput")
    t["w2"] = nc.dram_tensor("w2", [HID, 32], BF16, kind="ExternalInput")
    t["keep"] = nc.dram_tensor("keep", [128, 2 * C], BF16, kind="ExternalInput")
    t["cost"] = nc.dram_tensor("cost", [R, C], BF16, kind="ExternalInput")
    t["out"] = nc.dram_tensor("out", [R, E], F32, kind="ExternalOutput")
    # DRAM bounce buffers for cross-partition reshapes (DMA cannot stride
    # the SBUF partition dim; DRAM APs are unconstrained)
    t["fb"] = nc.dram_tensor("fbounce", [2, H, 128, C], BF16, kind="Internal")
    t["mb"] = nc.dram_tensor("mbounce", [R, H, C], BF16, kind="Internal")

    with tile.TileContext(nc) as tc:
        _kernel_body(tc, t)
    nc.compile()
    return nc


def _kernel_body(tc, t):
    nc = tc.nc
    with (
        tc.tile_pool(name="singles", bufs=1) as singles,
        tc.tile_pool(name="hp", bufs=4) as hpool,
        tc.tile_pool(name="yp", bufs=2) as ypool,
        tc.tile_pool(name="w1ps", bufs=3, space="PSUM") as w1ps,
        tc.tile_pool(name="w2ps", bufs=1, space="PSUM") as w2ps,
        tc.tile_pool(name="mmps", bufs=1, space="PSUM") as mmps,
    ):
        # ---- weights/constants to SBUF (one DMA each; issue on SP) ----
        ident = singles.tile([128, 128], BF16, tag="ident")
        make_identity(nc, ident)

        remb_sb = singles.tile([128, 4 * R], BF16, tag="remb")
        nc.sync.dma_start(out=remb_sb, in_=t["remb"].ap())
        wq_sb = singles.tile([128, 4 * E], BF16, tag="wq")
        nc.sync.dma_start(out=wq_sb, in_=t["wq"].ap())
        cemb_sb = singles.tile([128, 4 * C], BF16, tag="cemb")
        nc.sync.dma_start(out=cemb_sb, in_=t["cemb"].ap())
        wk_sb = singles.tile([128, 4 * E], BF16, tag="wk")
        nc.sync.dma_start(out=wk_sb, in_=t["wk"].ap())
        m9_sb = singles.tile([128, HID], BF16, tag="m9")
        nc.gpsimd.dma_start(out=m9_sb, in_=t["m9"].ap())
        wv_sb = singles.tile([128, 4 * E], BF16, tag="wv")
        nc.gpsimd.dma_start(out=wv_sb, in_=t["wv"].ap())
        w2_sb = singles.tile([HID, 32], BF16, tag="w2")
        nc.gpsimd.dma_start(out=w2_sb, in_=t["w2"].ap())
        keep_sb = singles.tile([128, 2 * C], BF16, tag="keep")
        nc.gpsimd.dma_start(out=keep_sb, in_=t["keep"].ap())

        # ---- QKV projections ----
        qt_sb = singles.tile([128, 4 * R], BF16, tag="qt")  # [hd, (m, r)]
        kt_sb = singles.tile([128, 4 * C], BF16, tag="kt")  # [hd, (m, c)]
        v_sb = singles.tile([128, 2 * E], BF16, tag="v")    # [c, (cc, h, d)]

        def proj(wsb, src_sb, dst_sb, n, eng_pair):
            for mm in range(2):
                ps = mmps.tile([128, 512], F32, tag="mm", name=f"qk{n}_{mm}")
                for sub in range(2):
                    m = 2 * mm + sub
                    for k in range(4):
                        nc.tensor.matmul(
                            ps[:, 256 * sub : 256 * (sub + 1)],
                            lhsT=wsb[:, 512 * k + 128 * m : 512 * k + 128 * (m + 1)],
                            rhs=src_sb[:, 256 * k : 256 * (k + 1)],
                            start=(k == 0), stop=(k == 3),
                        )
                eng = eng_pair[mm % 2]
                if eng is nc.scalar:
                    nc.scalar.copy(
                        out=dst_sb[:, 512 * mm : 512 * (mm + 1)], in_=ps
                    )
                else:
                    nc.vector.tensor_copy(
                        out=dst_sb[:, 512 * mm : 512 * (mm + 1)], in_=ps
                    )

        proj(wq_sb, remb_sb, qt_sb, "q", (nc.scalar, nc.vector))
        proj(wk_sb, cemb_sb, kt_sb, "k", (nc.vector, nc.scalar))
        # ---- dot scores -> F -> DRAM bounce -> S4 channel-major ----
        # S4[32g+ch, a*256 + c] = feat_ch[128*i + 32*g + a, c]; row 32g+8 = cost
        f_sb = [
            singles.tile([128, H * C], BF16, tag=f"f{i}", name=f"f{i}")
            for i in range(2)
        ]
        s4 = [
            singles.tile([128, 8192], BF16, tag=f"s4_{i}", name=f"s4_{i}")
            for i in range(2)
        ]
        for m in range(2):  # r chunk
            for j in range(4):       # head pair; separate PSUM banks so the
                pair = []            # two row-tiled matmuls can run concurrent
                for s in range(2):
                    ps = mmps.tile([128, 256], F32, tag="mm",
                                   name=f"dot{m}_{j}_{s}")
                    nc.tensor.matmul(
                        ps,
                        lhsT=qt_sb[64 * s : 64 * (s + 1),
                                   256 * j + 128 * m : 256 * j + 128 * (m + 1)],
                        rhs=kt_sb[64 * s : 64 * (s + 1), 256 * j : 256 * (j + 1)],
                        start=True, stop=True,
                        tile_position=(64 * s, 0),
                    )
                    pair.append(ps)
                for s in range(2):
                    h = 2 * j + s
                    dst = f_sb[m][:, 256 * h : 256 * (h + 1)]
                    if (j + s) % 2 == 0:
                        nc.scalar.copy(out=dst, in_=pair[s])
                    else:
                        nc.vector.tensor_copy(out=dst, in_=pair[s])
            # dump F channel-major: fb[m][ch, r_loc, c], then gather to S4
            nc.sync.dma_start(
                out=t["fb"].ap()[m].transpose([1, 0, 2]),
                in_=f_sb[m].rearrange("p (ch c) -> p ch c", ch=8),
            )
            for g in range(4):
                nc.sync.dma_start(
                    out=s4[m][32 * g : 32 * g + 8, :].rearrange(
                        "p (a b) -> p a b", a=32
                    ),
                    in_=t["fb"].ap()[m][:, 32 * g : 32 * (g + 1), :],
                )
                nc.gpsimd.dma_start(
                    out=s4[m][32 * g + 8 : 32 * g + 9, :],
                    in_=t["cost"].ap()[
                        128 * m + 32 * g : 128 * m + 32 * (g + 1), :
                    ],
                )
            if m == 0:
                # V projection fits here: not needed until AV, and the
                # m=0 bounce DMAs are in flight anyway
                for cc in range(2):
                    ps = mmps.tile([128, 512], F32, tag="mm", name=f"v{cc}")
                    for k in range(4):
                        nc.tensor.matmul(
                                    ps,
                            lhsT=cemb_sb[:, 256 * k + 128 * cc : 256 * k + 128 * (cc + 1)],
                            rhs=wv_sb[:, 512 * k : 512 * (k + 1)],
                            start=(k == 0), stop=(k == 3),
                        )
                    if cc == 0:
                        nc.scalar.copy(out=v_sb[:, 0:512], in_=ps)
                    else:
                        nc.vector.tensor_copy(out=v_sb[:, 512:1024], in_=ps)


        wo_sb = singles.tile([128, 4 * E], BF16, tag="wo")
        nc.gpsimd.dma_start(out=wo_sb, in_=t["wo"].ap())

        # ---- wave loop state ----
        w2p_fix = w2ps.tile([128, 512], F32, tag="w2p", name="w2p")
        eb = [
            singles.tile([128, 8192], BF16, tag=f"eb{i}", name=f"eb{i}")
            for i in range(2)
        ]
        l_sb = [
            singles.tile([128, H * C], BF16, tag=f"l{i}", name=f"l{i}")
            for i in range(2)
        ]
        pb = [
            singles.tile([128, H * C], BF16, tag=f"pb{i}", name=f"pb{i}")
            for i in range(2)
        ]
        # pt[i][c', (j, s, cc, r)] = p[128i+r, 128cc+c'] for head 2j+s
        pt_sb = [
            singles.tile([128, 2048], BF16, tag=f"pt{i}", name=f"pt{i}")
            for i in range(2)
        ]
        sums = [
            singles.tile([128, H], F32, tag=f"sums{i}", name=f"sums{i}")
            for i in range(2)
        ]
        recips = [
            singles.tile([128, H], F32, tag=f"recips{i}", name=f"recips{i}")
            for i in range(2)
        ]
        ot_sb = singles.tile([128, 4 * R], BF16, tag="ot")  # [e', (i, j, r)]

        def w1_wave(i, p, gg, wps):
            # unit p covers a-pair {2p, 2p+1}; gg selects row groups
            # (0,1) or (2,3).  Each matmul fills a full 512-f32 PSUM bank
            # so the two concurrent row-tiled matmuls never share a bank.
            for u in range(2):
                g = 2 * gg + u
                nc.tensor.matmul(
                    wps[:, 512 * u : 512 * (u + 1)],
                    lhsT=m9_sb[32 * g : 32 * g + 9, :],
                    rhs=s4[i][32 * g : 32 * g + 9, 512 * p : 512 * (p + 1)],
                    start=True, stop=True,
                    tile_position=(32 * g, 0),
                )

        def relu_pair(wpsA, wpsB, hA, hB):
            # one big ACT instr + two DVE instrs per unit: ACT pays its
            # ~250ns access bubble once, DVE's bubbles are cheaper
            nc.scalar.activation(
                out=hA[:, 0:XA], in_=wpsA[:, 0:XA], func=AF.Relu
            )
            nc.vector.tensor_scalar_max(
                out=hA[:, XA:1024], in0=wpsA[:, XA:1024], scalar1=0.0
            )
            nc.vector.tensor_scalar_max(out=hB, in0=wpsB, scalar1=0.0)

        def w2_wave(w2p, h_sb, gg):
            # 4 col-tiled matmuls share one bank partition-disjointly;
            # W2 is host-padded to M=32 with zero columns so each matmul
            # writes its full 32-partition band (no uninitialized PSUM)
            for u in range(2):
                g = 2 * gg + u
                nc.tensor.matmul(
                    w2p[32 * g : 32 * (g + 1), :],
                    lhsT=w2_sb,
                    rhs=h_sb[:, 512 * u : 512 * (u + 1)],
                    start=True, stop=True,
                    tile_position=(0, 32 * g),
                )

        def exp_evict(i, p, w2p):
            nc.scalar.activation(
                out=eb[i][:, 512 * p : 512 * (p + 1)], in_=w2p, func=AF.Exp
            )

        def mb_write(i, s):
            # eb[(g,h), (a'', c)] -> mb[128i+32g+16s+a'', h, c]
            # split across the Pool and Sync queues (issue cost ~0.7-1us each)
            for g in range(4):
                eng = nc.gpsimd if g % 2 == 0 else nc.sync
                eng.dma_start(
                    out=t["mb"].ap()[
                        128 * i + 32 * g + 16 * s : 128 * i + 32 * g + 16 * (s + 1)
                    ].transpose([1, 0, 2]),
                    in_=eb[i][32 * g : 32 * g + 8,
                              4096 * s : 4096 * (s + 1)].rearrange(
                        "p (a c) -> p a c", a=16
                    ),
                )

        def lsb_read(i):
            nc.sync.dma_start(
                out=l_sb[i].rearrange("p (h c) -> p h c", h=H),
                in_=t["mb"].ap()[128 * i : 128 * (i + 1)],
            )

        # ---- phase_c: softmax tail + AV + out projection for rchunk i,
        # expressed as a list of closures so it can interleave into the
        # other rchunk's wave loop ----
        def phase_items(i):
            items = []
            for hh in range(H):
                def stt(hh=hh):
                    hs = slice(C * hh, C * (hh + 1))
                    nc.vector.scalar_tensor_tensor(
                        out=pb[i][:, hs],
                        in0=l_sb[i][:, hs],
                        scalar=1.0,
                        in1=keep_sb[:, C * i : C * (i + 1)],
                        op0=ALU.mult,
                        op1=ALU.mult,
                        accum_out=sums[i][:, hh : hh + 1],
                    )
                items.append(stt)

            def recip():
                nc.vector.tensor_scalar_add(
                    out=sums[i], in0=sums[i], scalar1=1e-30
                )
                nc.vector.reciprocal(out=recips[i], in_=sums[i])
            items.append(recip)

            for hh in range(H):
                def scale(hh=hh):
                    hs = slice(C * hh, C * (hh + 1))
                    if hh % 2 == 0:
                        nc.scalar.activation(
                            out=pb[i][:, hs], in_=pb[i][:, hs], func=AF.Copy,
                            scale=recips[i][:, hh : hh + 1],
                        )
                    else:
                        nc.vector.tensor_scalar_mul(
                            out=pb[i][:, hs], in0=pb[i][:, hs],
                            scalar1=recips[i][:, hh : hh + 1],
                        )
                items.append(scale)

            for j in range(4):  # head pair: transposes then AV
                def transp(j=j):
                    tp = mmps.tile([128, 512], BF16, tag="mm", name=f"tp{i}_{j}")
                    for q in range(4):  # (s, cc)
                        s, cc = q // 2, q % 2
                        col = C * (2 * j + s) + 128 * cc
                        nc.tensor.transpose(
                            tp[:, 128 * q : 128 * (q + 1)],
                            in_=pb[i][:, col : col + 128],
                            identity=ident,
                        )
                    if j % 2 == 0:
                        nc.scalar.copy(
                            out=pt_sb[i][:, 512 * j : 512 * (j + 1)], in_=tp
                        )
                    else:
                        nc.vector.tensor_copy(
                            out=pt_sb[i][:, 512 * j : 512 * (j + 1)], in_=tp
                        )
                items.append(transp)

            av_state = {}
            def av_alloc():
                av_state["ps"] = mmps.tile(
                    [128, 512], F32, tag="mm", name=f"av{i}"
                )
            items.append(av_alloc)
            for j in range(4):
                def av(j=j):
                    ps = av_state["ps"]
                    for s in range(2):
                        h = 2 * j + s
                        for cc in range(2):
                            nc.tensor.matmul(
                                ps[64 * s : 64 * (s + 1), 128 * j : 128 * (j + 1)],
                                lhsT=v_sb[:, 512 * cc + 64 * h :
                                          512 * cc + 64 * (h + 1)],
                                rhs=pt_sb[i][:, 512 * j + 256 * s + 128 * cc :
                                             512 * j + 256 * s + 128 * (cc + 1)],
                                start=(cc == 0), stop=(cc == 1),
                            )
                items.append(av)

            def ot_evict():
                nc.vector.tensor_copy(
                    out=ot_sb[:, 512 * i : 512 * (i + 1)], in_=av_state["ps"]
                )
            items.append(ot_evict)

            def proj():
                ps = mmps.tile([128, 512], F32, tag="mm", name=f"yps{i}")
                for k in range(4):
                    nc.tensor.matmul(
                        ps,
                        lhsT=ot_sb[:, 512 * i + 128 * k : 512 * i + 128 * (k + 1)],
                        rhs=wo_sb[:, 512 * k : 512 * (k + 1)],
                        start=(k == 0), stop=(k == 3),
                    )
                y = ypool.tile([128, 512], F32, tag="y", name=f"y{i}")
                nc.scalar.copy(out=y, in_=ps)
                nc.sync.dma_start(
                    out=t["out"].ap()[128 * i : 128 * (i + 1), :], in_=y
                )
            items.append(proj)
            return items

        # ---- the wave loop, software-pipelined one unit deep:
        # iteration gp issues W1+relu(gp), then W2+exp(gp-1).  With
        # w1ps bufs=3 a relu has ~1.5 units before its PSUM slot is
        # reused, and the single w2ps slot has a full unit of slack,
        # so no cross-unit dependency cycle binds the cadence ----
        queue = []       # phase_c items to interleave
        h_of = {}        # gp -> (hA, hB)

        for gp in range(33):
            if gp < 32:
                i, p = divmod(gp, 16)
                wpsA = w1ps.tile([128, 1024], F32, tag="w1", name=f"wA{gp}")
                w1_wave(i, p, 0, wpsA)
                wpsB = w1ps.tile([128, 1024], F32, tag="w1", name=f"wB{gp}")
                w1_wave(i, p, 1, wpsB)
                hA = hpool.tile([128, 1024], BF16, tag="h", name=f"hA{gp}")
                hB = hpool.tile([128, 1024], BF16, tag="h", name=f"hB{gp}")
                relu_pair(wpsA, wpsB, hA, hB)
                h_of[gp] = (hA, hB)
            q = gp - 1
            if q >= 0:
                qi, qp = divmod(q, 16)
                hA, hB = h_of.pop(q)
                w2p = w2p_fix
                w2_wave(w2p, hA, 0)
                w2_wave(w2p, hB, 1)
                exp_evict(qi, qp, w2p)
                if qp == 7:
                    mb_write(qi, 0)
                elif qp == 15:
                    mb_write(qi, 1)
                    lsb_read(qi)
                    if qi == 0:
                        queue.extend(phase_items(0))
            if gp >= 17:
                for _ in range(3):
                    if queue:
                        queue.pop(0)()
        while queue:
            queue.pop(0)()
        for it in phase_items(1):
            it()


def _pack_rows(a):
    """[rows, cols] -> [128, (rows//128) * cols] with row = 128k + p."""
    rows, cols = a.shape
    k = rows // 128
    return np.ascontiguousarray(
        a.reshape(k, 128, cols).transpose(1, 0, 2).reshape(128, k * cols)
    )


def _prep_inputs(row_emb, col_emb, cost_mat, attn_mask, Wq, Wk, Wv, Wo, W1,
                 W2, alpha):
    bf = ml_dtypes.bfloat16
    alpha_v = np.asarray(alpha, np.float32).reshape(-1)  # [H]
    W1 = np.asarray(W1, np.float32)
    # M9 row 32g+h (h<8): W1[2h,:]/sqrt(D); row 32g+8: sum_h alpha_h*W1[2h+1,:]
    m9 = np.zeros((128, HID), np.float32)
    for g in range(4):
        for hh in range(H):
            m9[32 * g + hh] = W1[2 * hh] / np.sqrt(D)
        m9[32 * g + 8] = sum(alpha_v[hh] * W1[2 * hh + 1] for hh in range(H))
    shared = {
        "wq": _pack_rows(np.asarray(Wq, np.float32)).astype(bf),
        "wk": _pack_rows(np.asarray(Wk, np.float32)).astype(bf),
        "wv": _pack_rows(np.asarray(Wv, np.float32)).astype(bf),
        "wo": _pack_rows(np.asarray(Wo, np.float32)).astype(bf),
        "m9": m9.astype(bf),
        "w2": np.pad(np.asarray(W2, np.float32), ((0, 0), (0, 32 - H))).astype(bf),
    }
    in_maps = []
    for b in range(B):
        m = dict(shared)
        m["remb"] = _pack_rows(
            np.ascontiguousarray(np.asarray(row_emb[b], np.float32).T)
        ).astype(bf)
        m["cemb"] = _pack_rows(
            np.ascontiguousarray(np.asarray(col_emb[b], np.float32).T)
        ).astype(bf)
        m["cost"] = np.asarray(cost_mat[b, :, :, 0], np.float32).astype(bf)
        m["keep"] = _pack_rows(
            (~np.asarray(attn_mask[b])).astype(np.float32)
        ).astype(bf)
        in_maps.append(m)
    return in_maps


def kernel(**inputs) -> np.ndarray:
    global LAST_EXEC_NS
    if "nc" not in _CACHE:
        _CACHE["nc"] = _build()
    nc = _CACHE["nc"]
    in_maps = _prep_inputs(**inputs)
    trace = os.environ.get("KERNEL_TRACE", "0") == "1"
    res = run_bass_kernel_spmd(
        nc, in_maps, core_ids=list(range(NCORES)), trace=trace
    )
    LAST_EXEC_NS = res.exec_time_ns
    out = np.stack([np.asarray(res.results[b]["out"]) for b in range(B)])
    return out.astype(np.float32)
